# revision 1
# baseline (speedup 1.0000x reference)
"""Self-contained Trainium2 Bass kernel for the attention module:

    q = q_in @ Wq + bq ; k = k_in @ Wk + bk ; v = v_in @ Wv + bv
    out = softmax(q k^T / sqrt(64)) @ v           (B=4, S=4096, d=128, dk=64)

Sharding: 8 cores = 4 batches x 2 query-halves. Each core gets a 2048-row
q chunk plus the full K/V of its batch (the head dim is small, so K/V are
replicated across the 2 cores of a batch).

Math notes (all exact softmax identities, not approximations):
  - bk adds q_i . bk to every score of row i -> softmax-invariant, dropped.
  - bv shifts the output by attn-row-sum * bv = bv -> added on the host.
  - no max-subtraction: scores/8 are ~N(0, 1.3), |max| < ~7, exp is safe
    in fp32.

Device dataflow uses the S^T = [k, q] layout so exp(S^T) is directly the
lhsT/rhs pair for the P@V matmul (no transposes of the softmax matrix):
  QT [128, 2048] = wq_dup.T @ q_inT + bq  (wq_dup = [Wq | Wq] makes both
  partition halves hold Q^T, feeding the row-packed score matmuls)
  KT [128, 4096] = wk_dup.T @ k_inT
  V_aug [k, 65] per k-tile = [v_in Wv | ones]; the ones column makes the
  P@V matmul accumulate the softmax denominator as row 64 of O^T.
  Per q-block (512 q) x k-tile-pair: two 64-contract score matmuls packed
  into PE row-halves via tile_position (0,0)/(64,0); one ScalarE Exp over
  the 2-bank PSUM pair; two accumulating P@V matmuls into O^T [65, 512].
  Epilogue: PE-transpose O^T, divide by the denominator column on DVE,
  DMA out.
"""

import numpy as np
import ml_dtypes

import bass_rust
import concourse.bass as bass
import concourse.mybir as mybir
import concourse.tile as tile
from concourse import bass_utils
from concourse.masks import make_identity

MM_DTYPE = "bf16"   # "bf16" or "f32r"
ROW_PACK = True

SQ = 2048
SK = 4096
D = 128
DK = 64
QBLK = 512
NQB = SQ // QBLK
NKT = SK // 128
F32 = mybir.dt.float32
BF16 = mybir.dt.bfloat16
F32R = mybir.dt.float32r


def _fix_multiwait(nc):
    """This container's walrus rejects instructions carrying more than one
    sync-wait command; Tile attaches several. Split extras onto preceding
    same-engine NoOps (waits execute in queue order, so this is
    equivalent)."""
    for f in nc.m.functions:
        for bb in f.blocks:
            out = []
            changed = False
            for inst in bb.instructions:
                si = inst.sync_info
                waits = list(si.on_wait) if si is not None and si.on_wait else []
                if len(waits) > 1:
                    changed = True
                    for w in waits[:-1]:
                        nop = mybir.InstNoOp(
                            name=f"I-{nc.next_id()}", ins=[], outs=[]
                        )
                        nop.engine = inst.engine
                        nop.sync_info = bass_rust.SyncInfo(
                            on_wait=[w], on_update=[]
                        )
                        out.append(nop)
                    inst.sync_info = bass_rust.SyncInfo(
                        on_wait=[waits[-1]],
                        on_update=list(si.on_update or []),
                    )
                out.append(inst)
            if changed:
                bb.instructions = out


def _build(mm_dtype=MM_DTYPE, row_pack=ROW_PACK):
    sdt = F32R if mm_dtype == "f32r" else BF16

    nc = bass.Bass("TRN2", target_bir_lowering=False, debug=False)

    q_inT = nc.dram_tensor("q_inT", [D, SQ], sdt, kind="ExternalInput").ap()
    k_inT = nc.dram_tensor("k_inT", [D, SK], sdt, kind="ExternalInput").ap()
    v_inT = nc.dram_tensor("v_inT", [D, SK], sdt, kind="ExternalInput").ap()
    wq_dup = nc.dram_tensor("wq_dup", [D, D], sdt, kind="ExternalInput").ap()
    wk_dup = nc.dram_tensor("wk_dup", [D, D], sdt, kind="ExternalInput").ap()
    wv = nc.dram_tensor("wv", [D, DK], sdt, kind="ExternalInput").ap()
    bq_dup = nc.dram_tensor("bq_dup", [D, 1], F32, kind="ExternalInput").ap()
    o = nc.dram_tensor("o", [SQ, DK], F32, kind="ExternalOutput").ap()

    with tile.TileContext(nc) as tc:
        with (
            tc.sbuf_pool(name="cpool", bufs=1) as cpool,
            tc.sbuf_pool(name="inpool", bufs=3) as inpool,
            tc.sbuf_pool(name="wkpool", bufs=2) as wkpool,
        ):
            wq_sb = cpool.tile([D, D], sdt, name="wq_sb")
            wk_sb = cpool.tile([D, D], sdt, name="wk_sb")
            wv_sb = cpool.tile([D, DK], sdt, name="wv_sb")
            bq_sb = cpool.tile([D, 1], F32, name="bq_sb")
            ident = cpool.tile([DK + 1, DK + 1], F32, name="ident")
            nc.sync.dma_start(wq_sb, wq_dup)
            nc.sync.dma_start(wk_sb, wk_dup)
            nc.sync.dma_start(wv_sb, wv)
            nc.sync.dma_start(bq_sb, bq_dup)
            make_identity(nc, ident)

            QT = cpool.tile([D, SQ], sdt, name="QT")
            KT = cpool.tile([D, SK], sdt, name="KT")
            VA = cpool.tile([D, NKT * (DK + 1)], sdt, name="VA")
            ones_sb = cpool.tile([D, NKT], F32, name="ones_sb")
            nc.gpsimd.memset(ones_sb, 1.0)
            VA3 = VA.rearrange("p (t c) -> p t c", c=DK + 1)
            nc.vector.tensor_copy(VA3[:, :, 64], ones_sb)

            with tc.psum_pool(name="papool", bufs=2) as papool:
                for j in range(SQ // 512):
                    qin_c = inpool.tile([D, 512], sdt, name="qin_c", tag="qin")
                    nc.sync.dma_start(qin_c, q_inT[:, j * 512 : (j + 1) * 512])
                    q_ps = papool.tile([D, 512], F32, name="q_ps", tag="pps")
                    nc.tensor.matmul(q_ps, wq_sb, qin_c, start=True, stop=True)
                    nc.vector.tensor_scalar_add(
                        QT[:, j * 512 : (j + 1) * 512], q_ps, bq_sb
                    )
                for j in range(SK // 512):
                    kin_c = inpool.tile([D, 512], sdt, name="kin_c", tag="kin")
                    nc.sync.dma_start(kin_c, k_inT[:, j * 512 : (j + 1) * 512])
                    k_ps = papool.tile([D, 512], F32, name="k_ps", tag="pps")
                    nc.tensor.matmul(k_ps, wk_sb, kin_c, start=True, stop=True)
                    nc.vector.tensor_copy(KT[:, j * 512 : (j + 1) * 512], k_ps)
                for t in range(NKT):
                    vin_c = inpool.tile([D, 128], sdt, name="vin_c", tag="vin")
                    nc.sync.dma_start(vin_c, v_inT[:, t * 128 : (t + 1) * 128])
                    v_ps = papool.tile([D, DK], F32, name="v_ps", tag="vps")
                    nc.tensor.matmul(v_ps, vin_c, wv_sb, start=True, stop=True)
                    nc.vector.tensor_copy(
                        VA[:, t * (DK + 1) : t * (DK + 1) + DK], v_ps
                    )

            with (
                tc.psum_pool(name="spool", bufs=2) as spool,
                tc.psum_pool(name="opool", bufs=2) as opool,
                tc.psum_pool(name="tpool", bufs=2) as tpool,
            ):
                for qb in range(NQB):
                    qs = slice(qb * QBLK, (qb + 1) * QBLK)
                    o_ps = opool.tile([DK + 1, QBLK], F32, name="o_ps", tag="o")
                    for kp in range(NKT // 2):
                        kA, kB = 2 * kp, 2 * kp + 1
                        s_ps = spool.tile([D, 1024], F32, name="s_ps", tag="s")
                        if row_pack:
                            nc.tensor.matmul(
                                s_ps[:, 0:512],
                                KT[0:64, kA * 128 : (kA + 1) * 128],
                                QT[0:64, qs],
                                start=True, stop=True, tile_position=(0, 0),
                            )
                            nc.tensor.matmul(
                                s_ps[:, 512:1024],
                                KT[64:128, kB * 128 : (kB + 1) * 128],
                                QT[64:128, qs],
                                start=True, stop=True, tile_position=(64, 0),
                            )
                        else:
                            nc.tensor.matmul(
                                s_ps[:, 0:512],
                                KT[0:64, kA * 128 : (kA + 1) * 128],
                                QT[0:64, qs],
                                start=True, stop=True,
                            )
                            nc.tensor.matmul(
                                s_ps[:, 512:1024],
                                KT[0:64, kB * 128 : (kB + 1) * 128],
                                QT[0:64, qs],
                                start=True, stop=True,
                            )
                        pt = wkpool.tile([D, 1024], sdt, name="pt", tag="pt")
                        nc.scalar.activation(
                            pt, s_ps, mybir.ActivationFunctionType.Exp,
                            scale=0.125,
                        )
                        nc.tensor.matmul(
                            o_ps,
                            VA[:, kA * (DK + 1) : (kA + 1) * (DK + 1)],
                            pt[:, 0:512],
                            start=(kp == 0), stop=False,
                        )
                        nc.tensor.matmul(
                            o_ps,
                            VA[:, kB * (DK + 1) : (kB + 1) * (DK + 1)],
                            pt[:, 512:1024],
                            start=False, stop=(kp == NKT // 2 - 1),
                        )
                    o_sb = wkpool.tile([DK + 1, QBLK], F32, name="o_sb", tag="osb")
                    nc.vector.tensor_copy(o_sb, o_ps)
                    for t in range(QBLK // 128):
                        t_ps = tpool.tile([D, DK + 1], F32, name="t_ps", tag="t")
                        nc.tensor.transpose(
                            t_ps, o_sb[:, t * 128 : (t + 1) * 128], ident
                        )
                        rcp = wkpool.tile([D, 1], F32, name="rcp", tag="rcp")
                        nc.vector.reciprocal(rcp, t_ps[:, DK : DK + 1])
                        o_out = wkpool.tile([D, DK], F32, name="o_out", tag="oo")
                        nc.vector.tensor_scalar_mul(o_out, t_ps[:, 0:DK], rcp)
                        nc.sync.dma_start(
                            o[qb * QBLK + t * 128 : qb * QBLK + (t + 1) * 128, :],
                            o_out,
                        )
    _fix_multiwait(nc)
    return nc


_NC_CACHE = {}


def kernel(q_in, k_in, v_in, Wq, bq, Wk, bk, Wv, bv):
    q_in = np.asarray(q_in, np.float32)
    k_in = np.asarray(k_in, np.float32)
    v_in = np.asarray(v_in, np.float32)
    Wq = np.asarray(Wq, np.float32)
    bq = np.asarray(bq, np.float32)
    Wk = np.asarray(Wk, np.float32)
    Wv = np.asarray(Wv, np.float32)
    bv = np.asarray(bv, np.float32)

    np_dt = np.float32 if MM_DTYPE == "f32r" else ml_dtypes.bfloat16
    wq_dup = np.ascontiguousarray(np.concatenate([Wq, Wq], 1)).astype(np_dt)
    wk_dup = np.ascontiguousarray(np.concatenate([Wk, Wk], 1)).astype(np_dt)
    wv = np.ascontiguousarray(Wv).astype(np_dt)
    bq_dup = np.ascontiguousarray(
        np.concatenate([bq, bq])[:, None]
    ).astype(np.float32)

    in_maps = []
    for c in range(8):
        b, half = c // 2, c % 2
        rows = slice(half * SQ, (half + 1) * SQ)
        in_maps.append({
            "q_inT": np.ascontiguousarray(q_in[b, rows, :].T).astype(np_dt),
            "k_inT": np.ascontiguousarray(k_in[b].T).astype(np_dt),
            "v_inT": np.ascontiguousarray(v_in[b].T).astype(np_dt),
            "wq_dup": wq_dup, "wk_dup": wk_dup, "wv": wv, "bq_dup": bq_dup,
        })

    key = (MM_DTYPE, ROW_PACK)
    if key not in _NC_CACHE:
        _NC_CACHE[key] = _build()
    nc = _NC_CACHE[key]

    res = bass_utils.run_bass_kernel_spmd(nc, in_maps, list(range(8)))

    out = np.empty((4, 2 * SQ, DK), np.float32)
    for c in range(8):
        b, half = c // 2, c % 2
        out[b, half * SQ : (half + 1) * SQ, :] = res.results[c]["o"]
    return out + bv[None, None, :]


# revision 2
# speedup vs baseline: 1.1403x; 1.1403x over previous
"""Self-contained Trainium2 Bass kernel for the attention module:

    q = q_in @ Wq + bq ; k = k_in @ Wk + bk ; v = v_in @ Wv + bv
    out = softmax(q k^T / sqrt(64)) @ v           (B=4, S=4096, d=128, dk=64)

Sharding: 8 cores = 4 batches x 2 query-halves. Each core gets a 2048-row
q chunk plus the full K/V of its batch (the head dim is small, so K/V are
replicated across the 2 cores of a batch).

Math notes (all exact softmax identities, not approximations):
  - bk adds q_i . bk to every score of row i -> softmax-invariant, dropped.
  - bv shifts the output by attn-row-sum * bv = bv -> added on the host.
  - no max-subtraction: scores/8 are ~N(0, 1.3), |max| < ~7, exp is safe
    in fp32.

Device dataflow uses the S^T = [k, q] layout so exp(S^T) is directly the
lhsT/rhs pair for the P@V matmul (no transposes of the softmax matrix):
  QT [128, 2048] = wq_dup.T @ q_inT + bq  (wq_dup = [Wq | Wq] makes both
  partition halves hold Q^T, feeding the row-packed score matmuls)
  KT [128, 4096] = wk_dup.T @ k_inT
  V_aug [k, 65] per k-tile = [v_in Wv | ones]; the ones column makes the
  P@V matmul accumulate the softmax denominator as row 64 of O^T.
  Per q-block (512 q) x k-tile-pair: two 64-contract score matmuls packed
  into PE row-halves via tile_position (0,0)/(64,0); one ScalarE Exp over
  the 2-bank PSUM pair; two accumulating P@V matmuls into O^T [65, 512].
  Epilogue: PE-transpose O^T, divide by the denominator column on DVE,
  DMA out.
"""

import numpy as np
import ml_dtypes

import bass_rust
import concourse.bass as bass
import concourse.mybir as mybir
import concourse.tile as tile
from concourse import bass_utils
from concourse.masks import make_identity

MM_DTYPE = "bf16"   # "bf16" or "f32r"
ROW_PACK = True

SQ = 2048
SK = 4096
D = 128
DK = 64
QBLK = 512
NQB = SQ // QBLK
NKT = SK // 128
F32 = mybir.dt.float32
BF16 = mybir.dt.bfloat16
F32R = mybir.dt.float32r


def _fix_multiwait(nc):
    """This container's walrus rejects instructions carrying more than one
    sync-wait command; Tile attaches several. Split extras onto preceding
    same-engine NoOps (waits execute in queue order, so this is
    equivalent)."""
    for f in nc.m.functions:
        for bb in f.blocks:
            out = []
            changed = False
            for inst in bb.instructions:
                si = inst.sync_info
                waits = list(si.on_wait) if si is not None and si.on_wait else []
                if len(waits) > 1:
                    changed = True
                    for w in waits[:-1]:
                        nop = mybir.InstNoOp(
                            name=f"I-{nc.next_id()}", ins=[], outs=[]
                        )
                        nop.engine = inst.engine
                        nop.sync_info = bass_rust.SyncInfo(
                            on_wait=[w], on_update=[]
                        )
                        out.append(nop)
                    inst.sync_info = bass_rust.SyncInfo(
                        on_wait=[waits[-1]],
                        on_update=list(si.on_update or []),
                    )
                out.append(inst)
            if changed:
                bb.instructions = out


def _build(mm_dtype=MM_DTYPE, row_pack=ROW_PACK):
    sdt = F32R if mm_dtype == "f32r" else BF16

    nc = bass.Bass("TRN2", target_bir_lowering=False, debug=False)

    q_inT = nc.dram_tensor("q_inT", [D, SQ], sdt, kind="ExternalInput").ap()
    k_inT = nc.dram_tensor("k_inT", [D, SK], sdt, kind="ExternalInput").ap()
    v_inT = nc.dram_tensor("v_inT", [D, SK], sdt, kind="ExternalInput").ap()
    wq_dup = nc.dram_tensor("wq_dup", [D, D], sdt, kind="ExternalInput").ap()
    wk_dup = nc.dram_tensor("wk_dup", [D, D], sdt, kind="ExternalInput").ap()
    wv = nc.dram_tensor("wv", [D, DK], sdt, kind="ExternalInput").ap()
    bq_dup = nc.dram_tensor("bq_dup", [D, 1], F32, kind="ExternalInput").ap()
    o = nc.dram_tensor("o", [SQ, DK], F32, kind="ExternalOutput").ap()

    with tile.TileContext(nc) as tc:
        with (
            tc.sbuf_pool(name="cpool", bufs=1) as cpool,
            tc.sbuf_pool(name="inpool", bufs=3) as inpool,
            tc.sbuf_pool(name="wkpool", bufs=2) as wkpool,
        ):
            wq_sb = cpool.tile([D, D], sdt, name="wq_sb")
            wk_sb = cpool.tile([D, D], sdt, name="wk_sb")
            wv_sb = cpool.tile([D, DK], sdt, name="wv_sb")
            bq_sb = cpool.tile([D, 1], F32, name="bq_sb")
            ident = cpool.tile([DK + 1, DK + 1], F32, name="ident")
            nc.sync.dma_start(wq_sb, wq_dup)
            nc.sync.dma_start(wk_sb, wk_dup)
            nc.sync.dma_start(wv_sb, wv)
            nc.sync.dma_start(bq_sb, bq_dup)
            make_identity(nc, ident)

            warm = cpool.tile([D, 1], F32, name="warm")
            nc.scalar.activation(
                warm, bq_sb, mybir.ActivationFunctionType.Exp, scale=0.0
            )

            QT = cpool.tile([D, SQ], sdt, name="QT")
            KT = cpool.tile([D, SK], sdt, name="KT")
            VA = cpool.tile([D, NKT * (DK + 1)], sdt, name="VA")
            ones_sb = cpool.tile([D, NKT], F32, name="ones_sb")
            nc.gpsimd.memset(ones_sb, 1.0)
            VA3 = VA.rearrange("p (t c) -> p t c", c=DK + 1)
            nc.vector.tensor_copy(VA3[:, :, 64], ones_sb)

            # V first (big chunk loads), then Q, K; papool stays open so
            # attention matmuls can start as soon as their QT/KT slices land
            papool = tc.alloc_tile_pool(name="papool", bufs=2, space="PSUM")
            for j in range(SK // 1024):
                vbig = inpool.tile([D, 1024], sdt, name="vbig", tag="vin")
                nc.sync.dma_start(vbig, v_inT[:, j * 1024 : (j + 1) * 1024])
                for tt in range(8):
                    t = j * 8 + tt
                    v_ps = papool.tile([D, DK], F32, name="v_ps", tag="pps")
                    nc.tensor.matmul(
                        v_ps, vbig[:, tt * 128 : (tt + 1) * 128], wv_sb,
                        start=True, stop=True,
                    )
                    nc.vector.tensor_copy(
                        VA[:, t * (DK + 1) : t * (DK + 1) + DK], v_ps
                    )
            for j in range(SQ // 512):
                qin_c = inpool.tile([D, 512], sdt, name="qin_c", tag="qin")
                nc.sync.dma_start(qin_c, q_inT[:, j * 512 : (j + 1) * 512])
                q_ps = papool.tile([D, 512], F32, name="q_ps", tag="pps")
                nc.tensor.matmul(q_ps, wq_sb, qin_c, start=True, stop=True)
                nc.vector.tensor_scalar_add(
                    QT[:, j * 512 : (j + 1) * 512], q_ps, bq_sb
                )
            for j in range(SK // 512):
                kin_c = inpool.tile([D, 512], sdt, name="kin_c", tag="kin")
                nc.sync.dma_start(kin_c, k_inT[:, j * 512 : (j + 1) * 512])
                k_ps = papool.tile([D, 512], F32, name="k_ps", tag="pps")
                nc.tensor.matmul(k_ps, wk_sb, kin_c, start=True, stop=True)
                nc.vector.tensor_copy(KT[:, j * 512 : (j + 1) * 512], k_ps)

            with (
                tc.psum_pool(name="spool", bufs=2) as spool,
                tc.psum_pool(name="opool", bufs=1) as opool,
                tc.psum_pool(name="tpool", bufs=1) as tpool,
            ):
                for qb in range(NQB):
                    qs = slice(qb * QBLK, (qb + 1) * QBLK)
                    o_ps = opool.tile([DK + 1, QBLK], F32, name="o_ps", tag="o")
                    for kp in range(NKT // 2):
                        kA, kB = 2 * kp, 2 * kp + 1
                        s_ps = spool.tile([D, 1024], F32, name="s_ps", tag="s")
                        if row_pack:
                            nc.tensor.matmul(
                                s_ps[:, 0:512],
                                KT[0:64, kA * 128 : (kA + 1) * 128],
                                QT[0:64, qs],
                                start=True, stop=True, tile_position=(0, 0),
                            )
                            nc.tensor.matmul(
                                s_ps[:, 512:1024],
                                KT[64:128, kB * 128 : (kB + 1) * 128],
                                QT[64:128, qs],
                                start=True, stop=True, tile_position=(64, 0),
                            )
                        else:
                            nc.tensor.matmul(
                                s_ps[:, 0:512],
                                KT[0:64, kA * 128 : (kA + 1) * 128],
                                QT[0:64, qs],
                                start=True, stop=True,
                            )
                            nc.tensor.matmul(
                                s_ps[:, 512:1024],
                                KT[0:64, kB * 128 : (kB + 1) * 128],
                                QT[0:64, qs],
                                start=True, stop=True,
                            )
                        pt = wkpool.tile([D, 1024], sdt, name="pt", tag="pt")
                        nc.scalar.activation(
                            pt, s_ps, mybir.ActivationFunctionType.Exp,
                            scale=0.125,
                        )
                        nc.tensor.matmul(
                            o_ps,
                            VA[:, kA * (DK + 1) : (kA + 1) * (DK + 1)],
                            pt[:, 0:512],
                            start=(kp == 0), stop=False,
                        )
                        nc.tensor.matmul(
                            o_ps,
                            VA[:, kB * (DK + 1) : (kB + 1) * (DK + 1)],
                            pt[:, 512:1024],
                            start=False, stop=(kp == NKT // 2 - 1),
                        )
                    o_sb = wkpool.tile([DK + 1, QBLK], F32, name="o_sb", tag="osb")
                    nc.vector.tensor_copy(o_sb, o_ps)
                    for t in range(QBLK // 128):
                        t_ps = tpool.tile([D, DK + 1], F32, name="t_ps", tag="t")
                        nc.tensor.transpose(
                            t_ps, o_sb[:, t * 128 : (t + 1) * 128], ident
                        )
                        rcp = wkpool.tile([D, 1], F32, name="rcp", tag="rcp")
                        nc.vector.reciprocal(rcp, t_ps[:, DK : DK + 1])
                        o_out = wkpool.tile([D, DK], F32, name="o_out", tag="oo")
                        nc.vector.tensor_scalar_mul(o_out, t_ps[:, 0:DK], rcp)
                        nc.sync.dma_start(
                            o[qb * QBLK + t * 128 : qb * QBLK + (t + 1) * 128, :],
                            o_out,
                        )
            papool.release()
    _fix_multiwait(nc)
    return nc


_NC_CACHE = {}


def kernel(q_in, k_in, v_in, Wq, bq, Wk, bk, Wv, bv):
    q_in = np.asarray(q_in, np.float32)
    k_in = np.asarray(k_in, np.float32)
    v_in = np.asarray(v_in, np.float32)
    Wq = np.asarray(Wq, np.float32)
    bq = np.asarray(bq, np.float32)
    Wk = np.asarray(Wk, np.float32)
    Wv = np.asarray(Wv, np.float32)
    bv = np.asarray(bv, np.float32)

    np_dt = np.float32 if MM_DTYPE == "f32r" else ml_dtypes.bfloat16
    wq_dup = np.ascontiguousarray(np.concatenate([Wq, Wq], 1)).astype(np_dt)
    wk_dup = np.ascontiguousarray(np.concatenate([Wk, Wk], 1)).astype(np_dt)
    wv = np.ascontiguousarray(Wv).astype(np_dt)
    bq_dup = np.ascontiguousarray(
        np.concatenate([bq, bq])[:, None]
    ).astype(np.float32)

    in_maps = []
    for c in range(8):
        b, half = c // 2, c % 2
        rows = slice(half * SQ, (half + 1) * SQ)
        in_maps.append({
            "q_inT": np.ascontiguousarray(q_in[b, rows, :].T).astype(np_dt),
            "k_inT": np.ascontiguousarray(k_in[b].T).astype(np_dt),
            "v_inT": np.ascontiguousarray(v_in[b].T).astype(np_dt),
            "wq_dup": wq_dup, "wk_dup": wk_dup, "wv": wv, "bq_dup": bq_dup,
        })

    key = (MM_DTYPE, ROW_PACK)
    if key not in _NC_CACHE:
        _NC_CACHE[key] = _build()
    nc = _NC_CACHE[key]

    res = bass_utils.run_bass_kernel_spmd(nc, in_maps, list(range(8)))

    out = np.empty((4, 2 * SQ, DK), np.float32)
    for c in range(8):
        b, half = c // 2, c % 2
        out[b, half * SQ : (half + 1) * SQ, :] = res.results[c]["o"]
    return out + bv[None, None, :]
